# revision 11
# baseline (speedup 1.0000x reference)
"""NT-Xent (SimCLR contrastive) loss on Trainium2, sharded across 8 NeuronCores.

Each core computes a [512, 4096] row-slice of the similarity matrix
sim = zn_own^T . zn_all (fp8 DoubleRow matmuls, x16 fp8 scaling), with the
exp row-sums fused into ScalarE's activation accumulator and an exact
fp8-level diagonal recompute. Host sums the 8 scalar partials (the unshard
step). No host arithmetic beyond sharding/layout/dtype-cast of inputs and
summing the per-core partials.

v4 (vs the 62.7us baseline):
  - per-core column permutation: each core's zt is ordered
    [partner 512 | own 512 | rest 3072].  Row-sums are order-invariant, so
    the Gram covers the same set; the own rows' normalize factors are now a
    slice of block 0's rin (bit-identical math, so the diagonal recompute
    still cancels exactly), and the positives read the fp8 partner columns
    straight out of zn8[0].  This deletes the zown/zpr inputs (5.2 -> 4 MiB
    of input DMA) and the entire own/partner normalize chains.
  - half-block normalize conveyor, latency-ordered DVE queue, block-0 first.
  - PE warmers + density keep the clock gate at 2.4 GHz (measured: the Gram
    runs 215ns/matmul warm vs 427ns cold); all bulk elementwise work on DVE
    (GpSimd tensor ops are ~3.5x slower and poison concurrent DVE).
  - one activation-table load: Ln/Exp pinned via the bacc table-map patch.
"""

import numpy as np

B = 2048
D = 512
N2 = 2 * B              # 4096 total rows
NCORES = 8
RPC = N2 // NCORES      # 512 rows per core
KT = D // 128            # 4 contraction tiles
BLK = 1024              # column-block size
NBLK = N2 // BLK        # 4 blocks
TEMP = 0.1
SCALE = 1.0 / TEMP      # 10.0
FP8_SCALE = 16.0        # zn is stored as fp8(zn*16); sim256 = 256*sim
LN_FP8 = float(np.log(FP8_SCALE))
NWARM_A = 64            # PE warmers bridging the DMA head (9-16us)

_CACHE = {}


def _build():
    from concourse import bass, bacc, tile, mybir

    nc = bacc.Bacc("TRN2", target_bir_lowering=False, debug=False,
                   num_devices=NCORES)
    bf16 = mybir.dt.bfloat16
    f32 = mybir.dt.float32
    f8 = mybir.dt.float8e4
    F = mybir.ActivationFunctionType
    A = mybir.AluOpType
    AX = mybir.AxisListType
    DR = mybir.MatmulPerfMode.DoubleRow
    PSUM = bass.MemorySpace.PSUM

    # host-pre-permuted, half-major: zt[p, b, h, k, j] = z^T column
    # perm[b*1024 + h*512 + j], contraction row (k*128 + p), where perm =
    # [partner rows | own rows | rest].
    zt = nc.dram_tensor("zt", [128, NBLK, 2, KT, 512], bf16,
                        kind="ExternalInput").ap()
    out = nc.dram_tensor("out", [1, 1], f32, kind="ExternalOutput").ap()

    with tile.TileContext(nc) as tc:
        with (
            tc.tile_pool(name="sb", bufs=1) as sb,
            tc.tile_pool(name="wrk", bufs=2) as wrk,
            tc.tile_pool(name="wrk1", bufs=1) as wrk1,
            tc.tile_pool(name="psN", bufs=1, space=PSUM) as psN,
            tc.tile_pool(name="psO", bufs=2, space=PSUM) as psO,
            tc.tile_pool(name="psG", bufs=2, space=PSUM) as psG,
        ):
            ones = sb.tile([128, 128], bf16, tag="ones")
            nc.vector.memset(ones[:], 1.0)
            bias_ln16 = sb.tile([128, 1], f32, tag="b16")
            nc.vector.memset(bias_ln16[:], LN_FP8)
            bias_10 = sb.tile([128, 1], f32, tag="b10")
            nc.vector.memset(bias_10[:], SCALE)

            # ---- input DMAs on the sync HWDGE queue; the own half (b0 h1)
            # first since it gates the Gram lhs.
            zb = [sb.tile([128, 2, KT, 512], bf16, tag=f"zt{b}",
                          name=f"zb{b}") for b in range(NBLK)]
            nc.sync.dma_start(out=zb[0][:, 1], in_=zt[:, 0, 1])
            nc.sync.dma_start(out=zb[0][:, 0], in_=zt[:, 0, 0])
            nc.sync.dma_start(out=zb[1][:, 0], in_=zt[:, 1, 0])
            nc.sync.dma_start(out=zb[1][:, 1], in_=zt[:, 1, 1])
            nc.sync.dma_start(out=zb[2][:], in_=zt[:, 2])
            nc.sync.dma_start(out=zb[3][:], in_=zt[:, 3])

            # ---- PE warmers: ramp the clock gate during the DMA head
            warm = psO.tile([128, 512], f32, tag="pd", name="warmA")
            for _ in range(NWARM_A):
                nc.tensor.matmul(warm[:, 0:128], ones[:], ones[:],
                                 start=True, stop=True)

            zn16 = [sb.tile([128, 2, KT, 512], bf16, tag=f"zn16_{b}",
                            name=f"zn16_{b}") for b in range(NBLK)]
            zn8 = [sb.tile([128, 2, KT, 512], f8, tag=f"zn8_{b}",
                           name=f"zn8_{b}") for b in range(NBLK)]
            zno = sb.tile([128, KT, RPC], f8, tag="zno")
            rin = [None] * NBLK
            psS = [None] * NBLK
            sq_t = [None] * NBLK
            for b in range(NBLK):
                sq_t[b] = wrk.tile([128, 2, KT, 512], bf16,
                                   tag="sq01" if b < 2 else "sq23",
                                   name=f"sq{b}")
                psS[b] = psN.tile([128, BLK], f32, tag="ssq", name=f"psS{b}")

            def sq_half(b, h):
                nc.vector.tensor_tensor(sq_t[b][:, h], zb[b][:, h],
                                        zb[b][:, h], A.mult)

            def sq_full(b):
                nc.vector.tensor_tensor(sq_t[b][:], zb[b][:], zb[b][:],
                                        A.mult)

            def ssq_half(b, h):
                for k in range(KT):
                    nc.tensor.matmul(psS[b][:, h * 512:(h + 1) * 512],
                                     ones[:], sq_t[b][:, h, k, :],
                                     start=(k == 0), stop=(k == KT - 1))

            def rsqrt_block(b):
                lns = wrk.tile([128, BLK], f32, tag="lns", name=f"lns{b}")
                nc.scalar.activation(lns[:], psS[b][:], F.Ln)
                rin[b] = wrk1.tile([128, BLK], bf16, tag=f"rin{b}",
                                   name=f"rin{b}")
                nc.scalar.activation(rin[b][:], lns[:], F.Exp, scale=-0.5,
                                     bias=bias_ln16[:])

            def mult_half(b, h):
                nc.vector.tensor_tensor(
                    zn16[b][:, h], zb[b][:, h],
                    rin[b][:, h * 512:(h + 1) * 512]
                    .unsqueeze(1).broadcast_to([128, KT, 512]), A.mult)
                nc.gpsimd.dma_start(out=zn8[b][:, h], in_=zn16[b][:, h])

            # ---- conveyor, latency-ordered.  Block 0's own half gates
            # zno (and with it the whole Gram), so its square/ssq/rsqrt
            # run at half granularity and everything else follows.
            sq_half(0, 1)          # own half first
            sq_half(0, 0)
            ssq_half(0, 1)
            ssq_half(0, 0)
            lns0 = wrk.tile([128, BLK], f32, tag="lns", name="lns0")
            rin[0] = wrk1.tile([128, BLK], bf16, tag="rin0", name="rin0")
            nc.scalar.activation(lns0[:, 512:1024], psS[0][:, 512:1024], F.Ln)
            nc.scalar.activation(rin[0][:, 512:1024], lns0[:, 512:1024],
                                 F.Exp, scale=-0.5, bias=bias_ln16[:])
            fillr = psO.tile([128, 512], f32, tag="pd", name="fillr")
            for _ in range(8):
                nc.tensor.matmul(fillr[:], ones[:], rin[0][:, 512:1024],
                                 start=True, stop=True)
            nc.scalar.activation(lns0[:, 0:512], psS[0][:, 0:512], F.Ln)
            nc.scalar.activation(rin[0][:, 0:512], lns0[:, 0:512],
                                 F.Exp, scale=-0.5, bias=bias_ln16[:])

            # DVE: zno (= the Gram's own-column fp8 too; zn8[0] h1 IS zno),
            # then the block-0 partner-half multiply.
            nc.vector.tensor_tensor(
                zno[:], zb[0][:, 1],
                rin[0][:, 512:1024].unsqueeze(1).broadcast_to([128, KT, 512]),
                A.mult)
            mult_half(0, 0)
            # PE clock-keeper fillers: become ready with zno and soak the
            # PE idle window before the Gram so the clock gate stays up.
            fill = psO.tile([128, 512], f32, tag="pd", name="fill")
            for _ in range(8):
                nc.tensor.matmul(fill[:], ones[:], zno[:, 0, :],
                                 start=True, stop=True)

            # ---------- Gram + fused exp row-sums ----------
            rowp = sb.tile([128, 4, NBLK], f32, tag="rowp")
            scr_n = [0]

            def gram_group(b, m):
                pm = psG.tile([128, BLK], f32, tag="mm", name=f"pm{b}_{m}")
                lhsT0 = zno[:, 0:2, m * 128:(m + 1) * 128]
                lhsT1 = zno[:, 2:4, m * 128:(m + 1) * 128]
                for h in range(2):
                    hs = slice(h * 512, (h + 1) * 512)
                    rhs = zno if (b == 0 and h == 1) else zn8[b][:, h]
                    nc.tensor.matmul(pm[:, hs], lhsT0, rhs[:, 0:2, :],
                                     start=True, stop=False, perf_mode=DR)
                    nc.tensor.matmul(pm[:, hs], lhsT1, rhs[:, 2:4, :],
                                     start=False, stop=True, perf_mode=DR)
                scr = wrk.tile([128, BLK], bf16, tag="scr",
                               name=f"scr{scr_n[0]}")
                scr_n[0] += 1
                nc.scalar.activation(
                    scr[:], pm[:], F.Exp,
                    scale=SCALE / (FP8_SCALE ** 2),
                    accum_out=rowp[:, m, b:b + 1])

            # first two Gram groups outrank ssq1 on PE so the exp stream
            # opens as soon as zn8[0] lands
            gram_group(0, 0)
            gram_group(0, 1)

            # remaining squares + rsqrts, all ahead of the exp stream in
            # ScalarE priority; sq2 fills DVE's wait for rin1, sq3 runs
            # before block 2's multiplies so no late chain cascades.
            sq_half(1, 0)
            sq_half(1, 1)
            ssq_half(1, 0)
            ssq_half(1, 1)
            rsqrt_block(1)
            sq_full(2)
            ssq_half(2, 0)
            ssq_half(2, 1)
            rsqrt_block(2)
            mult_half(1, 0)
            mult_half(1, 1)
            sq_full(3)
            ssq_half(3, 0)
            ssq_half(3, 1)
            rsqrt_block(3)
            mult_half(2, 0)
            mult_half(2, 1)
            mult_half(3, 0)
            mult_half(3, 1)

            for b in range(NBLK):
                for m in range(4):
                    if not (b == 0 and m < 2):
                        gram_group(b, m)

            # ---- diagonal recompute (exact fp8-level) + positives ----
            prd = wrk.tile([128, KT, RPC], bf16, tag="prod", name="prd")
            nc.vector.tensor_tensor(prd[:], zno[:], zno[:], A.mult)
            dg = psO.tile([128, 512], f32, tag="pd", name="dg")
            for k in range(KT):
                nc.tensor.matmul(dg[0:1, :], ones[:, 0:1], prd[:, k, :],
                                 start=(k == 0), stop=(k == KT - 1))
            diag_row = sb.tile([1, RPC], bf16, tag="diagrow")
            nc.vector.tensor_scalar_add(diag_row[:], dg[0:1, :],
                                        -FP8_SCALE ** 2)
            dt = psO.tile([128, 512], f32, tag="pd", name="dt")
            for m in range(4):
                nc.tensor.matmul(dt[:, m * 128:(m + 1) * 128],
                                 diag_row[0:1, m * 128:(m + 1) * 128],
                                 ones[0:1, :], start=True, stop=True)
            diag_part = sb.tile([128, 4], f32, tag="diagp")
            for m in range(4):
                nc.vector.tensor_copy(diag_part[:, m:m + 1],
                                      dt[:, m * 128:m * 128 + 1])
            dexp = sb.tile([128, 4], f32, tag="dexp")
            nc.scalar.activation(dexp[:], diag_part[:], F.Exp,
                                 scale=SCALE / (FP8_SCALE ** 2),
                                 bias=bias_10[:])

            # positives: fp8 own x fp8 partner (block 0, half 0)
            prp = wrk.tile([128, KT, RPC], bf16, tag="prod", name="prp")
            nc.vector.tensor_tensor(prp[:], zno[:], zn8[0][:, 0], A.mult)
            pp = psO.tile([128, 512], f32, tag="pd", name="pp")
            for k in range(KT):
                nc.tensor.matmul(pp[:], ones[:], prp[:, k, :],
                                 start=(k == 0), stop=(k == KT - 1))
            pos_red = sb.tile([128, 1], f32, tag="posr")
            nc.vector.tensor_reduce(pos_red[:], pp[:], AX.X, A.add)

            # ---- finale: partial = sum_r ln(Z_r) - 10 * sum_r pos_r ----
            zs = sb.tile([128, 4], f32, tag="zs")
            nc.vector.tensor_reduce(zs[:], rowp[:], AX.X, A.add)
            zarg = sb.tile([128, 4], f32, tag="zarg")
            nc.vector.tensor_tensor(zarg[:], zs[:], dexp[:], A.subtract)
            logz = sb.tile([128, 5], f32, tag="logz")
            nc.scalar.activation(logz[:, 0:4], zarg[:], F.Ln)
            nc.vector.tensor_scalar_mul(
                logz[:, 4:5], pos_red[:], -SCALE / (FP8_SCALE ** 2) / 128.0)
            red1 = sb.tile([128, 1], f32, tag="red1")
            nc.vector.tensor_reduce(red1[:], logz[:], AX.X, A.add)
            fin = sb.tile([1, 1], f32, tag="fin")
            nc.gpsimd.tensor_reduce(fin[:], red1[:], AX.C, A.add)
            nc.sync.dma_start(out=out, in_=fin[:])

    from concourse import bacc as _bacc_mod

    orig_tables = _bacc_mod.get_activation_tables

    def _filtered(arch):
        tables = orig_tables(arch)
        keep = "natural_log_exp_and_others"
        F = mybir.ActivationFunctionType
        if (keep in tables and F.Exp in tables[keep]
                and F.Ln in tables[keep]):
            for name, fns in tables.items():
                if name != keep:
                    fns.discard(F.Exp)
                    fns.discard(F.Ln)
        return tables

    _bacc_mod.get_activation_tables = _filtered
    try:
        nc.compile()
    finally:
        _bacc_mod.get_activation_tables = orig_tables
    return nc


def _get_nc():
    if "nc" not in _CACHE:
        _CACHE["nc"] = _build()
    return _CACHE["nc"]


def _in_maps(z_i, z_j):
    import ml_dtypes

    z = np.concatenate(
        [np.asarray(z_i, np.float32), np.asarray(z_j, np.float32)], axis=0)
    zt = np.ascontiguousarray(z.T).astype(ml_dtypes.bfloat16)  # [D, N2]

    maps = []
    all_idx = np.arange(N2)
    for c in range(NCORES):
        o = c * RPC
        po = (o + B) % N2
        own = all_idx[o:o + RPC]
        par = all_idx[po:po + RPC]
        rest = np.setdiff1d(all_idx, np.concatenate([own, par]))
        perm = np.concatenate([par, own, rest])
        ztp = zt[:, perm]                       # [D, N2], permuted columns
        ztH = np.ascontiguousarray(
            ztp.reshape(KT, 128, NBLK, 2, 512).transpose(1, 2, 3, 0, 4))
        maps.append({"zt": ztH})
    return maps


def _run(z_i, z_j, trace=False):
    from concourse.bass_utils import run_bass_kernel_spmd

    nc = _get_nc()
    return run_bass_kernel_spmd(nc, _in_maps(z_i, z_j), list(range(NCORES)),
                                trace=trace)


def kernel(z_i, z_j):
    res = _run(z_i, z_j, trace=False)
    total = sum(float(r["out"][0, 0]) for r in res.results)
    return np.float32(total / N2)


# revision 13
# speedup vs baseline: 1.0799x; 1.0799x over previous
"""NT-Xent (SimCLR contrastive) loss on Trainium2, sharded across 8 NeuronCores.

Each core computes a [512, 4096] row-slice of the similarity matrix
sim = zn_own^T . zn_all (fp8 DoubleRow matmuls, x16 fp8 scaling), with the
exp row-sums fused into ScalarE's activation accumulator and an exact
fp8-level diagonal recompute. Host sums the 8 scalar partials (the unshard
step). No host arithmetic beyond sharding/layout/dtype-cast of inputs and
summing the per-core partials.

v8, ~55.0us (vs the ~62us baseline):
  - per-core column permutation: each core's zt is ordered
    [partner 512 | own 512 | rest 3072].  Row-sums are order-invariant, so
    the Gram covers the same set; the own rows' normalize factors are now a
    slice of block 0's rin (bit-identical math, so the diagonal recompute
    still cancels exactly), and the positives read the fp8 partner columns
    straight out of zn8[0].  This deletes the zown/zpr inputs (5.2 -> 4 MiB
    of input DMA) and the entire own/partner normalize chains.
  - half-block normalize conveyor, latency-ordered DVE queue, block-0 first;
    zno doubles as zn8[0]'s own half (one multiply+cast dropped from the
    critical path); each block's normalize multiply is queued before any
    later block's square so no late chain cascades.
  - all block rsqrts outrank the exp stream in ScalarE priority, and the
    first two Gram groups outrank ssq1 on PE: the exp stream opens at ~25us
    and runs gapless to ~50us.
  - PE warmers (64) + zno/rin-gated filler matmuls keep the clock gate at
    2.4 GHz (measured: Gram matmuls run 215ns warm vs 427ns cold); all bulk
    elementwise work on DVE (GpSimd tensor ops are ~3.5x slower and poison
    concurrent DVE throughput ~6x).
  - one activation-table load: Ln/Exp pinned via the bacc table-map patch.
"""

import numpy as np

B = 2048
D = 512
N2 = 2 * B              # 4096 total rows
NCORES = 8
RPC = N2 // NCORES      # 512 rows per core
KT = D // 128            # 4 contraction tiles
BLK = 1024              # column-block size
NBLK = N2 // BLK        # 4 blocks
TEMP = 0.1
SCALE = 1.0 / TEMP      # 10.0
FP8_SCALE = 16.0        # zn is stored as fp8(zn*16); sim256 = 256*sim
LN_FP8 = float(np.log(FP8_SCALE))
NWARM_A = 64            # PE warmers bridging the DMA head (9-16us)

_CACHE = {}


def _build():
    from concourse import bass, bacc, tile, mybir

    nc = bacc.Bacc("TRN2", target_bir_lowering=False, debug=False,
                   num_devices=NCORES)
    bf16 = mybir.dt.bfloat16
    f32 = mybir.dt.float32
    f8 = mybir.dt.float8e4
    F = mybir.ActivationFunctionType
    A = mybir.AluOpType
    AX = mybir.AxisListType
    DR = mybir.MatmulPerfMode.DoubleRow
    PSUM = bass.MemorySpace.PSUM

    # host-pre-permuted, half-major: zt[p, b, h, k, j] = z^T column
    # perm[b*1024 + h*512 + j], contraction row (k*128 + p), where perm =
    # [partner rows | own rows | rest].
    zt = nc.dram_tensor("zt", [128, NBLK, 2, KT, 512], bf16,
                        kind="ExternalInput").ap()
    out = nc.dram_tensor("out", [1, 1], f32, kind="ExternalOutput").ap()

    with tile.TileContext(nc) as tc:
        with (
            tc.tile_pool(name="sb", bufs=1) as sb,
            tc.tile_pool(name="wrk", bufs=2) as wrk,
            tc.tile_pool(name="wrk1", bufs=1) as wrk1,
            tc.tile_pool(name="psN", bufs=1, space=PSUM) as psN,
            tc.tile_pool(name="psO", bufs=2, space=PSUM) as psO,
            tc.tile_pool(name="psG", bufs=2, space=PSUM) as psG,
        ):
            ones = sb.tile([128, 128], bf16, tag="ones")
            nc.vector.memset(ones[:], 1.0)
            bias_ln16 = sb.tile([128, 1], f32, tag="b16")
            nc.vector.memset(bias_ln16[:], LN_FP8)
            bias_10 = sb.tile([128, 1], f32, tag="b10")
            nc.vector.memset(bias_10[:], SCALE)

            # ---- input DMAs on the sync HWDGE queue; the own half (b0 h1)
            # first since it gates the Gram lhs.
            zb = [sb.tile([128, 2, KT, 512], bf16, tag=f"zt{b}",
                          name=f"zb{b}") for b in range(NBLK)]
            nc.sync.dma_start(out=zb[0][:, 1], in_=zt[:, 0, 1])
            nc.sync.dma_start(out=zb[0][:, 0], in_=zt[:, 0, 0])
            nc.sync.dma_start(out=zb[1][:, 0], in_=zt[:, 1, 0])
            nc.sync.dma_start(out=zb[1][:, 1], in_=zt[:, 1, 1])
            nc.sync.dma_start(out=zb[2][:], in_=zt[:, 2])
            nc.sync.dma_start(out=zb[3][:], in_=zt[:, 3])

            # ---- PE warmers: ramp the clock gate during the DMA head
            warm = psO.tile([128, 512], f32, tag="pd", name="warmA")
            for _ in range(NWARM_A):
                nc.tensor.matmul(warm[:, 0:128], ones[:], ones[:],
                                 start=True, stop=True)

            zn16 = [sb.tile([128, 2, KT, 512], bf16, tag=f"zn16_{b}",
                            name=f"zn16_{b}") for b in range(NBLK)]
            zn8 = [sb.tile([128, 2, KT, 512], f8, tag=f"zn8_{b}",
                           name=f"zn8_{b}") for b in range(NBLK)]
            zno = sb.tile([128, KT, RPC], f8, tag="zno")
            rin = [None] * NBLK
            psS = [None] * NBLK
            sq_t = [None] * NBLK
            for b in range(NBLK):
                sq_t[b] = wrk.tile([128, 2, KT, 512], bf16,
                                   tag="sq01" if b < 2 else "sq23",
                                   name=f"sq{b}")
                psS[b] = psN.tile([128, BLK], f32, tag="ssq", name=f"psS{b}")

            def sq_half(b, h):
                nc.vector.tensor_tensor(sq_t[b][:, h], zb[b][:, h],
                                        zb[b][:, h], A.mult)

            def sq_full(b):
                nc.vector.tensor_tensor(sq_t[b][:], zb[b][:], zb[b][:],
                                        A.mult)

            def ssq_half(b, h):
                for k in range(KT):
                    nc.tensor.matmul(psS[b][:, h * 512:(h + 1) * 512],
                                     ones[:], sq_t[b][:, h, k, :],
                                     start=(k == 0), stop=(k == KT - 1))

            def rsqrt_block(b):
                lns = wrk.tile([128, BLK], f32, tag="lns", name=f"lns{b}")
                nc.scalar.activation(lns[:], psS[b][:], F.Ln)
                rin[b] = wrk1.tile([128, BLK], bf16, tag=f"rin{b}",
                                   name=f"rin{b}")
                nc.scalar.activation(rin[b][:], lns[:], F.Exp, scale=-0.5,
                                     bias=bias_ln16[:])

            def mult_half(b, h):
                nc.vector.tensor_tensor(
                    zn16[b][:, h], zb[b][:, h],
                    rin[b][:, h * 512:(h + 1) * 512]
                    .unsqueeze(1).broadcast_to([128, KT, 512]), A.mult)
                nc.gpsimd.dma_start(out=zn8[b][:, h], in_=zn16[b][:, h])

            # ---- conveyor, latency-ordered.  Block 0's own half gates
            # zno (and with it the whole Gram), so its square/ssq/rsqrt
            # run at half granularity and everything else follows.
            sq_half(0, 1)          # own half first
            sq_half(0, 0)
            ssq_half(0, 1)
            ssq_half(0, 0)
            lns0 = wrk.tile([128, BLK], f32, tag="lns", name="lns0")
            rin[0] = wrk1.tile([128, BLK], bf16, tag="rin0", name="rin0")
            nc.scalar.activation(lns0[:, 512:1024], psS[0][:, 512:1024], F.Ln)
            nc.scalar.activation(rin[0][:, 512:1024], lns0[:, 512:1024],
                                 F.Exp, scale=-0.5, bias=bias_ln16[:])
            fillr = psO.tile([128, 512], f32, tag="pd", name="fillr")
            for _ in range(8):
                nc.tensor.matmul(fillr[:], ones[:], rin[0][:, 512:1024],
                                 start=True, stop=True)
            nc.scalar.activation(lns0[:, 0:512], psS[0][:, 0:512], F.Ln)
            nc.scalar.activation(rin[0][:, 0:512], lns0[:, 0:512],
                                 F.Exp, scale=-0.5, bias=bias_ln16[:])

            # DVE: zno (= the Gram's own-column fp8 too; zn8[0] h1 IS zno),
            # then the block-0 partner-half multiply.
            nc.vector.tensor_tensor(
                zno[:], zb[0][:, 1],
                rin[0][:, 512:1024].unsqueeze(1).broadcast_to([128, KT, 512]),
                A.mult)
            mult_half(0, 0)
            # PE clock-keeper fillers: become ready with zno and soak the
            # PE idle window before the Gram so the clock gate stays up.
            fill = psO.tile([128, 512], f32, tag="pd", name="fill")
            for _ in range(8):
                nc.tensor.matmul(fill[:], ones[:], zno[:, 0, :],
                                 start=True, stop=True)

            # ---------- Gram + fused exp row-sums ----------
            rowp = sb.tile([128, 4, NBLK], f32, tag="rowp")
            scr_n = [0]

            def gram_group(b, m):
                pm = psG.tile([128, BLK], f32, tag="mm", name=f"pm{b}_{m}")
                lhsT0 = zno[:, 0:2, m * 128:(m + 1) * 128]
                lhsT1 = zno[:, 2:4, m * 128:(m + 1) * 128]
                for h in range(2):
                    hs = slice(h * 512, (h + 1) * 512)
                    rhs = zno if (b == 0 and h == 1) else zn8[b][:, h]
                    nc.tensor.matmul(pm[:, hs], lhsT0, rhs[:, 0:2, :],
                                     start=True, stop=False, perf_mode=DR)
                    nc.tensor.matmul(pm[:, hs], lhsT1, rhs[:, 2:4, :],
                                     start=False, stop=True, perf_mode=DR)
                scr = wrk.tile([128, BLK], bf16, tag="scr",
                               name=f"scr{scr_n[0]}")
                scr_n[0] += 1
                nc.scalar.activation(
                    scr[:], pm[:], F.Exp,
                    scale=SCALE / (FP8_SCALE ** 2),
                    accum_out=rowp[:, m, b:b + 1])

            # first two Gram groups outrank ssq1 on PE so the exp stream
            # opens as soon as zn8[0] lands
            gram_group(0, 0)
            gram_group(0, 1)

            # remaining squares + rsqrts, all ahead of the exp stream in
            # ScalarE priority; sq2 fills DVE's wait for rin1, sq3 runs
            # before block 2's multiplies so no late chain cascades.
            sq_half(1, 0)
            sq_half(1, 1)
            ssq_half(1, 0)
            ssq_half(1, 1)
            rsqrt_block(1)
            mult_half(1, 0)
            mult_half(1, 1)
            sq_full(2)
            ssq_half(2, 0)
            ssq_half(2, 1)
            rsqrt_block(2)
            sq_full(3)
            ssq_half(3, 0)
            ssq_half(3, 1)
            rsqrt_block(3)
            mult_half(2, 0)
            mult_half(2, 1)
            mult_half(3, 0)
            mult_half(3, 1)

            for b in range(NBLK):
                for m in range(4):
                    if not (b == 0 and m < 2):
                        gram_group(b, m)

            # ---- diagonal recompute (exact fp8-level) + positives ----
            prd = wrk.tile([128, KT, RPC], bf16, tag="prod", name="prd")
            nc.vector.tensor_tensor(prd[:], zno[:], zno[:], A.mult)
            dg = psO.tile([128, 512], f32, tag="pd", name="dg")
            for k in range(KT):
                nc.tensor.matmul(dg[0:1, :], ones[:, 0:1], prd[:, k, :],
                                 start=(k == 0), stop=(k == KT - 1))
            diag_row = sb.tile([1, RPC], bf16, tag="diagrow")
            nc.vector.tensor_scalar_add(diag_row[:], dg[0:1, :],
                                        -FP8_SCALE ** 2)
            dt = psO.tile([128, 512], f32, tag="pd", name="dt")
            for m in range(4):
                nc.tensor.matmul(dt[:, m * 128:(m + 1) * 128],
                                 diag_row[0:1, m * 128:(m + 1) * 128],
                                 ones[0:1, :], start=True, stop=True)
            diag_part = sb.tile([128, 4], f32, tag="diagp")
            for m in range(4):
                nc.vector.tensor_copy(diag_part[:, m:m + 1],
                                      dt[:, m * 128:m * 128 + 1])
            dexp = sb.tile([128, 4], f32, tag="dexp")
            nc.scalar.activation(dexp[:], diag_part[:], F.Exp,
                                 scale=SCALE / (FP8_SCALE ** 2),
                                 bias=bias_10[:])

            # positives: fp8 own x fp8 partner (block 0, half 0)
            prp = wrk.tile([128, KT, RPC], bf16, tag="prod", name="prp")
            nc.vector.tensor_tensor(prp[:], zno[:], zn8[0][:, 0], A.mult)
            pp = psO.tile([128, 512], f32, tag="pd", name="pp")
            for k in range(KT):
                nc.tensor.matmul(pp[:], ones[:], prp[:, k, :],
                                 start=(k == 0), stop=(k == KT - 1))
            pos_red = sb.tile([128, 1], f32, tag="posr")
            nc.vector.tensor_reduce(pos_red[:], pp[:], AX.X, A.add)

            # ---- finale: partial = sum_r ln(Z_r) - 10 * sum_r pos_r ----
            zs = sb.tile([128, 4], f32, tag="zs")
            nc.vector.tensor_reduce(zs[:], rowp[:], AX.X, A.add)
            zarg = sb.tile([128, 4], f32, tag="zarg")
            nc.vector.tensor_tensor(zarg[:], zs[:], dexp[:], A.subtract)
            logz = sb.tile([128, 5], f32, tag="logz")
            nc.scalar.activation(logz[:, 0:4], zarg[:], F.Ln)
            nc.vector.tensor_scalar_mul(
                logz[:, 4:5], pos_red[:], -SCALE / (FP8_SCALE ** 2) / 128.0)
            red1 = sb.tile([128, 1], f32, tag="red1")
            nc.vector.tensor_reduce(red1[:], logz[:], AX.X, A.add)
            fin = sb.tile([1, 1], f32, tag="fin")
            nc.gpsimd.tensor_reduce(fin[:], red1[:], AX.C, A.add)
            nc.sync.dma_start(out=out, in_=fin[:])

    from concourse import bacc as _bacc_mod

    orig_tables = _bacc_mod.get_activation_tables

    def _filtered(arch):
        tables = orig_tables(arch)
        keep = "natural_log_exp_and_others"
        F = mybir.ActivationFunctionType
        if (keep in tables and F.Exp in tables[keep]
                and F.Ln in tables[keep]):
            for name, fns in tables.items():
                if name != keep:
                    fns.discard(F.Exp)
                    fns.discard(F.Ln)
        return tables

    _bacc_mod.get_activation_tables = _filtered
    try:
        nc.compile()
    finally:
        _bacc_mod.get_activation_tables = orig_tables
    return nc


def _get_nc():
    if "nc" not in _CACHE:
        _CACHE["nc"] = _build()
    return _CACHE["nc"]


def _in_maps(z_i, z_j):
    import ml_dtypes

    z = np.concatenate(
        [np.asarray(z_i, np.float32), np.asarray(z_j, np.float32)], axis=0)
    zt = np.ascontiguousarray(z.T).astype(ml_dtypes.bfloat16)  # [D, N2]

    maps = []
    all_idx = np.arange(N2)
    for c in range(NCORES):
        o = c * RPC
        po = (o + B) % N2
        own = all_idx[o:o + RPC]
        par = all_idx[po:po + RPC]
        rest = np.setdiff1d(all_idx, np.concatenate([own, par]))
        perm = np.concatenate([par, own, rest])
        ztp = zt[:, perm]                       # [D, N2], permuted columns
        ztH = np.ascontiguousarray(
            ztp.reshape(KT, 128, NBLK, 2, 512).transpose(1, 2, 3, 0, 4))
        maps.append({"zt": ztH})
    return maps


def _run(z_i, z_j, trace=False):
    from concourse.bass_utils import run_bass_kernel_spmd

    nc = _get_nc()
    return run_bass_kernel_spmd(nc, _in_maps(z_i, z_j), list(range(NCORES)),
                                trace=trace)


def kernel(z_i, z_j):
    res = _run(z_i, z_j, trace=False)
    total = sum(float(r["out"][0, 0]) for r in res.results)
    return np.float32(total / N2)
